# revision 19
# baseline (speedup 1.0000x reference)
"""KDLoss kernel for 8 TRN2 NeuronCores — single-launch fp8 symmetric gram.

loss = sqrt(N * || Tn@Tn.T - Rn@Rn.T ||_F^2 + 1e-5), Tn/Rn row-normalized.

Rewrites the Frobenius norm through the stacked matrix C = [Tn | Rn]
(N x 2D) and its symmetric gram M = C^T C (4096 x 4096):

  || Tn Tn^T - Rn Rn^T ||_F^2 = sum_{ij} s_i s_j M_ij^2 ,
  s = (+1 on the Tn half, -1 on the Rn half).

M is symmetric, so the device computes only cyclic block-diagonals
d = 0..15 of the 32x32 grid of 128x128 blocks (512 blocks, vs 768 for
three dense grams). Every core runs the same SPMD program on a rotated
view of the slabs: core c's local slab j holds global slab (4c + j) mod
32, and it computes blocks (a, a+d) for a in 0..3, d in 0..15. The 16
d=16 blocks (the diagonal of the Tn^T Rn cross gram) would be computed
twice under this rotation, so the host does them once in full f32
instead. Host maps each accumulator column back to its global block and
applies the +-1/x2 weights in float64.

Normalization, the fp8e4 cast (scaled by 16 to center the dynamic
range; end-to-end rel err ~7e-4), slab packing, and the final
weighted-reduce + sqrt run on the host; the device does only the gram
contractions (fp8e4 DoubleRow matmuls into f32 PSUM, ~855 ns per block
at the sustained ~150 TF/s fp8 rate) and per-block square-sums (scalar
Square + vector reduce). Input DMAs are k-quartered and triggered from
the scalar queue so the tensor engine starts ~11 us into the NEFF and
then runs gapless.
"""

import sys

if "/opt/trn_rl_repo" not in sys.path:
    sys.path.insert(0, "/opt/trn_rl_repo")

from contextlib import ExitStack

import ml_dtypes
import numpy as np

import concourse.bacc as bacc
import concourse.tile as tile
from concourse import mybir
from concourse.bass_utils import run_bass_kernel_spmd

N_CORES = 8
N, D = 4096, 2048
NB = 32                  # slabs of 128 gram columns (2D / 128)
JL = 20                  # local slabs per core: a in 0..3, b = a+d, d <= 16
NKP = 16                 # 32 k-tiles as 16 DoubleRow pairs
SCALE = 16.0             # pre-quantization scale to center fp8e4 range
EPS_NORM = 1e-12
EPS_LOSS = 1e-05
F32 = mybir.dt.float32
FP8 = mybir.dt.float8e4

# chain layout: (a, chunk, jstart, width); all rhs groups sit inside one
# 4-slab chunk so each chain depends on exactly one input DMA. Chains are
# emitted phase-major — chains of one (phase, a) interleave their matmuls so
# consecutive matmuls share the same stationary lhsT slab.
PHASES = [(0, 1), (2, 3, 4)]


def _chain(a, g):
    # device covers diagonals d = 0..15 only; the 16 d=16 blocks (the
    # diagonal of the Tn^T Rn cross gram) are cheaper to do exactly on host
    # than to compute twice under the SPMD rotation
    if g == 0:
        return (a, g, a, 4 - a)
    if g == 4:
        return (a, g, 16, a)
    return (a, g, 4 * g, 4)


# last phase runs a descending so the final chain drain is the smallest
A_ORDER = {0: (0, 1, 2, 3), 1: (3, 2, 1, 0)}
CHAINS = [
    _chain(a, g)
    for pi, phase in enumerate(PHASES)
    for a in A_ORDER[pi]
    for g in phase
    if _chain(a, g)[3] > 0
]
ACC_COLS = sum(w for _, _, _, w in CHAINS)  # 64


def build_gram():
    nc = bacc.Bacc("TRN2", target_bir_lowering=False, num_devices=N_CORES)
    # (p, chunk, k, jc): lane p reads 16 KB contiguous per chunk
    slabs_in = nc.dram_tensor("slabs", [128, 5, NB, 512], FP8, kind="ExternalInput").ap()
    acc_out = nc.dram_tensor("acc", [128, ACC_COLS], F32, kind="ExternalOutput").ap()

    with tile.TileContext(nc) as tc, ExitStack() as ctx:
        data = ctx.enter_context(tc.tile_pool(name="data", bufs=1))
        psum = ctx.enter_context(tc.tile_pool(name="psum", bufs=7, space="PSUM"))
        red = ctx.enter_context(tc.tile_pool(name="red", bufs=4))

        # chunks split into k-quarters so the first matmul starts after 1/4 of
        # a chunk lands; DMA emission order matches compute order. Triggered
        # from the scalar queue (HWDGE), which finishes its startup ucode load
        # ahead of the sync queue's first DMA slot.
        pieces = {}
        order = [(i, q) for q in range(4) for i in (0, 1)] + [
            (i, q) for q in range(4) for i in (2, 3, 4)
        ]
        for i, q in order:
            t = data.tile([128, NB // 4, 512], FP8, tag=f"c{i}q{q}")
            nc.scalar.dma_start(t[:], slabs_in[:, i, 8 * q : 8 * q + 8, :])
            pieces[i, q] = t
        acc = data.tile([128, ACC_COLS], F32, tag="acc")

        col = 0
        acc_done = 0
        for pi, phase in enumerate(PHASES):
            for a in A_ORDER[pi]:
                chains = []
                for g in phase:
                    _, _, js, w = _chain(a, g)
                    if w == 0:
                        continue
                    ps = psum.tile([128, 512], F32, tag="ps")
                    chains.append((ps, g, js - 4 * g, w))
                for kk in range(NKP):
                    q, kk2 = divmod(kk, NKP // 4)
                    lhsT = pieces[0, q][:, 2 * kk2 : 2 * kk2 + 2, 128 * a : 128 * (a + 1)]
                    for ps, g, jloc, w in chains:
                        nc.tensor.matmul(
                            ps[:, : w * 128],
                            lhsT=lhsT,
                            rhs=pieces[g, q][
                                :, 2 * kk2 : 2 * kk2 + 2, 128 * jloc : 128 * (jloc + w)
                            ],
                            start=(kk == 0),
                            stop=(kk == NKP - 1),
                            perf_mode=mybir.MatmulPerfMode.DoubleRow,
                        )
                for ps, g, jloc, w in chains:
                    sq = red.tile([128, 512], F32, tag="sq")
                    nc.scalar.activation(
                        sq[:, : w * 128],
                        ps[:, : w * 128],
                        mybir.ActivationFunctionType.Square,
                    )
                    nc.vector.tensor_reduce(
                        acc[:, col : col + w],
                        sq[:, : w * 128].rearrange("p (j c) -> p j c", c=128),
                        axis=mybir.AxisListType.X,
                        op=mybir.AluOpType.add,
                    )
                    col += w
                # in the last phase, ship each a's columns as they finish
                if pi == len(PHASES) - 1:
                    nc.sync.dma_start(acc_out[:, acc_done:col], acc[:, acc_done:col])
                    acc_done = col
            if acc_done < col:
                nc.sync.dma_start(acc_out[:, acc_done:col], acc[:, acc_done:col])
                acc_done = col
    nc.compile()
    return nc


_CACHE = {}


def _get(name, builder):
    if name not in _CACHE:
        _CACHE[name] = builder()
    return _CACHE[name]


def pack_inputs(results, targets):
    """Host: normalize rows, cast to fp8, pack per-core rotated slab arrays.

    Also returns the d=16 cross-gram diagonal square-sum (full f32 GEMM,
    f64 reduce), which the device scheme would otherwise compute twice.
    """
    tn = np.maximum(np.sqrt(np.einsum("ij,ij->i", targets, targets)), EPS_NORM)
    rn = np.maximum(np.sqrt(np.einsum("ij,ij->i", results, results)), EPS_NORM)
    C = np.empty((N, 2 * D), dtype=np.float32)
    np.multiply(targets, (SCALE / tn)[:, None], out=C[:, :D])
    np.multiply(results, (SCALE / rn)[:, None], out=C[:, D:])
    Ct = C[:, :D].reshape(N, 16, 128).transpose(1, 2, 0)
    Cr = C[:, D:].reshape(N, 16, 128).transpose(1, 0, 2)
    cross = np.matmul(Ct, Cr)  # [16, 128, 128] = Tn_i^T @ Rn_i, scaled
    host_sq = float((cross.astype(np.float64) ** 2).sum())
    C8 = C.astype(ml_dtypes.float8_e4m3)
    # [n, col] -> [p, j, k, c] with n = 128k + p, col = 128j + c
    S = C8.reshape(NB, 128, NB, 128).transpose(1, 2, 0, 3)
    in_maps = []
    for c in range(N_CORES):
        jidx = [(4 * c + j) % NB for j in range(JL)]
        # [p, 20j, k, c] -> [p, 5chunk, k, 4j*128c]
        sc = (
            S[:, jidx, :, :]
            .reshape(128, 5, 4, NB, 128)
            .transpose(0, 1, 3, 2, 4)
            .reshape(128, 5, NB, 512)
        )
        in_maps.append({"slabs": np.ascontiguousarray(sc)})
    return in_maps, host_sq


def finish(res, host_sq):
    """Host: weighted f64 reduction of per-block square sums + final sqrt."""
    total = -2.0 * host_sq  # d=16 blocks are all cross-half (sign -1, x2)
    for c in range(N_CORES):
        colsum = res[c]["acc"].astype(np.float64).sum(axis=0)
        col = 0
        for a, g, js, w in CHAINS:
            i_g = 4 * c + a
            for t in range(w):
                j = js + t
                d = j - a
                j_g = (4 * c + j) % NB
                sign = 1.0 if (i_g < 16) == (j_g < 16) else -1.0
                mult = 1.0 if d == 0 else 2.0
                total += sign * mult * colsum[col]
                col += 1
    sq = total / SCALE**4
    return np.float32(np.sqrt(sq * N + EPS_LOSS))


def kernel(results, targets):
    results = np.asarray(results, dtype=np.float32)
    targets = np.asarray(targets, dtype=np.float32)
    in_maps, host_sq = pack_inputs(results, targets)
    nc = _get("gram", build_gram)
    res = run_bass_kernel_spmd(nc, in_maps, list(range(N_CORES))).results
    return finish(res, host_sq)


# revision 22
# speedup vs baseline: 1.0151x; 1.0151x over previous
"""KDLoss kernel for 8 TRN2 NeuronCores — single-launch fp8 symmetric gram.

loss = sqrt(N * || Tn@Tn.T - Rn@Rn.T ||_F^2 + 1e-5), Tn/Rn row-normalized.

Rewrites the Frobenius norm through the stacked matrix C = [Tn | Rn]
(N x 2D) and its symmetric gram M = C^T C (4096 x 4096):

  || Tn Tn^T - Rn Rn^T ||_F^2 = sum_{ij} s_i s_j M_ij^2 ,
  s = (+1 on the Tn half, -1 on the Rn half).

M is symmetric, so the device computes only cyclic block-diagonals
d = 0..15 of the 32x32 grid of 128x128 blocks (512 blocks, vs 768 for
three dense grams). Every core runs the same SPMD program on a rotated
view of the slabs: core c's local slab j holds global slab (4c + j) mod
32, and it computes blocks (a, a+d) for a in 0..3, d in 0..15. The 16
d=16 blocks (the diagonal of the Tn^T Rn cross gram) would be computed
twice under this rotation, so the host does them once in full f32
instead. Host maps each accumulator column back to its global block and
applies the +-1/x2 weights in float64.

Normalization, the fp8e4 cast (scaled by 16 to center the dynamic
range; end-to-end rel err ~7e-4), slab packing, and the final
weighted-reduce + sqrt run on the host; the device does only the gram
contractions (fp8e4 DoubleRow matmuls into f32 PSUM, ~855 ns per block
at the sustained ~150 TF/s fp8 rate) and per-block square-sums (scalar
Square + vector reduce). Input DMAs are k-quartered and triggered from
the scalar queue so the tensor engine starts ~11 us into the NEFF and
then runs gapless.
"""

import sys

if "/opt/trn_rl_repo" not in sys.path:
    sys.path.insert(0, "/opt/trn_rl_repo")

from contextlib import ExitStack

import ml_dtypes
import numpy as np

import concourse.bacc as bacc
import concourse.tile as tile
from concourse import mybir
from concourse.bass_utils import run_bass_kernel_spmd

N_CORES = 8
N, D = 4096, 2048
NB = 32                  # slabs of 128 gram columns (2D / 128)
JL = 20                  # local slabs per core: a in 0..3, b = a+d, d <= 16
NKP = 16                 # 32 k-tiles as 16 DoubleRow pairs
SCALE = 16.0             # pre-quantization scale to center fp8e4 range
EPS_NORM = 1e-12
EPS_LOSS = 1e-05
F32 = mybir.dt.float32
FP8 = mybir.dt.float8e4

# chain layout: (a, g, jstart, width). The device covers diagonals d = 0..15
# only (the 16 d=16 blocks — the diagonal of the Tn^T Rn cross gram — are
# cheaper to do exactly on host than to compute twice under the SPMD
# rotation), so per a the rhs windows are [a+4g .. a+4g+3]: every chain is a
# full 512-wide matmul stream. Windows for g<2 live in slab-tile A (slabs
# 0..11), for g>=2 in tile B (slabs 8..19); the 4-slab overlap is loaded
# twice, which is cheap next to the matmul time it saves. Chains of one
# (phase, a) interleave their matmuls so consecutive matmuls share the same
# stationary lhsT slab.
PHASES = [(0, 1), (2, 3)]


def _chain(a, g):
    return (a, g, a + 4 * g, 4)


A_ORDER = {0: (0, 1, 2, 3), 1: (3, 2, 1, 0)}
CHAINS = [
    _chain(a, g) for pi, phase in enumerate(PHASES) for a in A_ORDER[pi] for g in phase
]
ACC_COLS = sum(w for _, _, _, w in CHAINS)  # 64
TILE_SLABS = {0: list(range(12)), 1: list(range(8, 20))}  # A, B


def build_gram():
    nc = bacc.Bacc("TRN2", target_bir_lowering=False, num_devices=N_CORES)
    # (p, tile, k, jc): lane p reads 12 slabs x 128 contiguous per (tile, k)
    slabs_in = nc.dram_tensor(
        "slabs", [128, 2, NB, 12 * 128], FP8, kind="ExternalInput"
    ).ap()
    acc_out = nc.dram_tensor("acc", [128, ACC_COLS], F32, kind="ExternalOutput").ap()

    with tile.TileContext(nc) as tc, ExitStack() as ctx:
        data = ctx.enter_context(tc.tile_pool(name="data", bufs=1))
        psum = ctx.enter_context(tc.tile_pool(name="psum", bufs=7, space="PSUM"))
        red = ctx.enter_context(tc.tile_pool(name="red", bufs=4))

        # each slab tile is split into k-eighths so the first matmul starts
        # after 1/8 of tile A lands; DMA emission order matches compute order.
        # Triggered from the scalar queue (HWDGE), which finishes its startup
        # ucode load ahead of the sync queue's first DMA slot.
        pieces = {}
        for ti, q in [(t, q) for t in range(2) for q in range(8)]:
            p = data.tile([128, NB // 8, 12 * 128], FP8, tag=f"t{ti}q{q}")
            nc.scalar.dma_start(p[:], slabs_in[:, ti, 4 * q : 4 * q + 4, :])
            pieces[ti, q] = p
        acc = data.tile([128, ACC_COLS], F32, tag="acc")

        col = 0
        acc_done = 0
        for pi, phase in enumerate(PHASES):
            for a in A_ORDER[pi]:
                chains = []
                for g in phase:
                    _, _, js, w = _chain(a, g)
                    jloc = js - 8 * pi  # tile B starts at slab 8
                    ps = psum.tile([128, 512], F32, tag="ps")
                    chains.append((ps, g, jloc, w))
                for kk in range(NKP):
                    q, kk2 = divmod(kk, 2)
                    # lhsT slab a always lives in tile A
                    lhsT = pieces[0, q][:, 2 * kk2 : 2 * kk2 + 2, 128 * a : 128 * (a + 1)]
                    for ps, g, jloc, w in chains:
                        nc.tensor.matmul(
                            ps[:, : w * 128],
                            lhsT=lhsT,
                            rhs=pieces[pi, q][
                                :, 2 * kk2 : 2 * kk2 + 2, 128 * jloc : 128 * (jloc + w)
                            ],
                            start=(kk == 0),
                            stop=(kk == NKP - 1),
                            perf_mode=mybir.MatmulPerfMode.DoubleRow,
                        )
                for ps, g, jloc, w in chains:
                    sq = red.tile([128, 512], F32, tag="sq")
                    nc.scalar.activation(
                        sq[:, : w * 128],
                        ps[:, : w * 128],
                        mybir.ActivationFunctionType.Square,
                    )
                    nc.vector.tensor_reduce(
                        acc[:, col : col + w],
                        sq[:, : w * 128].rearrange("p (j c) -> p j c", c=128),
                        axis=mybir.AxisListType.X,
                        op=mybir.AluOpType.add,
                    )
                    col += w
                # in the last phase, ship each a's columns as they finish
                if pi == len(PHASES) - 1:
                    nc.sync.dma_start(acc_out[:, acc_done:col], acc[:, acc_done:col])
                    acc_done = col
            if acc_done < col:
                nc.sync.dma_start(acc_out[:, acc_done:col], acc[:, acc_done:col])
                acc_done = col
    nc.compile()
    return nc


_CACHE = {}


def _get(name, builder):
    if name not in _CACHE:
        _CACHE[name] = builder()
    return _CACHE[name]


def pack_inputs(results, targets):
    """Host: normalize rows, cast to fp8, pack per-core rotated slab arrays.

    Also returns the d=16 cross-gram diagonal square-sum (full f32 GEMM,
    f64 reduce), which the device scheme would otherwise compute twice.
    """
    tn = np.maximum(np.sqrt(np.einsum("ij,ij->i", targets, targets)), EPS_NORM)
    rn = np.maximum(np.sqrt(np.einsum("ij,ij->i", results, results)), EPS_NORM)
    C = np.empty((N, 2 * D), dtype=np.float32)
    np.multiply(targets, (SCALE / tn)[:, None], out=C[:, :D])
    np.multiply(results, (SCALE / rn)[:, None], out=C[:, D:])
    Ct = C[:, :D].reshape(N, 16, 128).transpose(1, 2, 0)
    Cr = C[:, D:].reshape(N, 16, 128).transpose(1, 0, 2)
    cross = np.matmul(Ct, Cr)  # [16, 128, 128] = Tn_i^T @ Rn_i, scaled
    host_sq = float((cross.astype(np.float64) ** 2).sum())
    C8 = C.astype(ml_dtypes.float8_e4m3)
    # [n, col] -> [p, j, k, c] with n = 128k + p, col = 128j + c
    S = C8.reshape(NB, 128, NB, 128).transpose(1, 2, 0, 3)
    in_maps = []
    for c in range(N_CORES):
        jidx = [(4 * c + j) % NB for j in range(JL)]
        rot = S[:, jidx, :, :]  # [p, 20j, k, c]
        tiles = [
            rot[:, sl, :, :].transpose(0, 2, 1, 3).reshape(128, NB, 12 * 128)
            for sl in (TILE_SLABS[0], TILE_SLABS[1])
        ]
        in_maps.append({"slabs": np.ascontiguousarray(np.stack(tiles, axis=1))})
    return in_maps, host_sq


def finish(res, host_sq):
    """Host: weighted f64 reduction of per-block square sums + final sqrt."""
    total = -2.0 * host_sq  # d=16 blocks are all cross-half (sign -1, x2)
    for c in range(N_CORES):
        colsum = res[c]["acc"].astype(np.float64).sum(axis=0)
        col = 0
        for a, g, js, w in CHAINS:
            i_g = 4 * c + a
            for t in range(w):
                j = js + t
                d = j - a
                j_g = (4 * c + j) % NB
                sign = 1.0 if (i_g < 16) == (j_g < 16) else -1.0
                mult = 1.0 if d == 0 else 2.0
                total += sign * mult * colsum[col]
                col += 1
    sq = total / SCALE**4
    return np.float32(np.sqrt(sq * N + EPS_LOSS))


def kernel(results, targets):
    results = np.asarray(results, dtype=np.float32)
    targets = np.asarray(targets, dtype=np.float32)
    in_maps, host_sq = pack_inputs(results, targets)
    nc = _get("gram", build_gram)
    res = run_bass_kernel_spmd(nc, in_maps, list(range(N_CORES))).results
    return finish(res, host_sq)


# revision 24
# speedup vs baseline: 1.0181x; 1.0030x over previous
"""KDLoss kernel for 8 TRN2 NeuronCores — single-launch fp8 symmetric gram.

loss = sqrt(N * || Tn@Tn.T - Rn@Rn.T ||_F^2 + 1e-5), Tn/Rn row-normalized.

Rewrites the Frobenius norm through the stacked matrix C = [Tn | Rn]
(N x 2D) and its symmetric gram M = C^T C (4096 x 4096):

  || Tn Tn^T - Rn Rn^T ||_F^2 = sum_{ij} s_i s_j M_ij^2 ,
  s = (+1 on the Tn half, -1 on the Rn half).

M is symmetric, so the device computes only cyclic block-diagonals
d = 0..15 of the 32x32 grid of 128x128 blocks (512 blocks, vs 768 for
three dense grams). Every core runs the same SPMD program on a rotated
view of the slabs: core c's local slab j holds global slab (4c + j) mod
32, and it computes blocks (a, a+d) for a in 0..3, d in 0..15. The 16
d=16 blocks (the diagonal of the Tn^T Rn cross gram) would be computed
twice under this rotation, so the host does them once in full f32
instead. Host maps each accumulator column back to its global block and
applies the +-1/x2 weights in float64.

Normalization, the fp8e4 cast (scaled by 16 to center the dynamic
range; end-to-end rel err ~7e-4), slab packing, and the final
weighted-reduce + sqrt run on the host; the device does only the gram
contractions (fp8e4 DoubleRow matmuls into f32 PSUM, ~855 ns per block
at the sustained ~150 TF/s fp8 rate) and per-block square-sums (scalar
Square + vector reduce). Input DMAs are k-quartered and triggered from
the scalar queue so the tensor engine starts ~11 us into the NEFF and
then runs gapless.
"""

import sys

if "/opt/trn_rl_repo" not in sys.path:
    sys.path.insert(0, "/opt/trn_rl_repo")

from contextlib import ExitStack

import ml_dtypes
import numpy as np

import concourse.bacc as bacc
import concourse.tile as tile
from concourse import mybir
from concourse.bass_utils import run_bass_kernel_spmd

N_CORES = 8
N, D = 4096, 2048
NB = 32                  # slabs of 128 gram columns (2D / 128)
JL = 20                  # local slabs per core: a in 0..3, b = a+d, d <= 16
NKP = 16                 # 32 k-tiles as 16 DoubleRow pairs
SCALE = 16.0             # pre-quantization scale to center fp8e4 range
EPS_NORM = 1e-12
EPS_LOSS = 1e-05
F32 = mybir.dt.float32
FP8 = mybir.dt.float8e4

# chain layout: (a, g, jstart, width). The device covers diagonals d = 0..15
# only (the 16 d=16 blocks — the diagonal of the Tn^T Rn cross gram — are
# cheaper to do exactly on host than to compute twice under the SPMD
# rotation), so per a the rhs windows are [a+4g .. a+4g+3]: every chain is a
# full 512-wide matmul stream. Windows for g<2 live in slab-tile A (slabs
# 0..11), for g>=2 in tile B (slabs 8..19); the 4-slab overlap is loaded
# twice, which is cheap next to the matmul time it saves. Chains of one
# (phase, a) interleave their matmuls so consecutive matmuls share the same
# stationary lhsT slab.
PHASES = [(0, 1), (2, 3)]


def _chain(a, g):
    return (a, g, a + 4 * g, 4)


A_ORDER = {0: (0, 1, 2, 3), 1: (3, 2, 1, 0)}
CHAINS = [
    _chain(a, g) for pi, phase in enumerate(PHASES) for a in A_ORDER[pi] for g in phase
]
ACC_COLS = sum(w for _, _, _, w in CHAINS)  # 64
TILE_SLABS = {0: list(range(12)), 1: list(range(8, 20))}  # A, B


def build_gram():
    nc = bacc.Bacc("TRN2", target_bir_lowering=False, num_devices=N_CORES)
    # (p, tile, k, jc): lane p reads 12 slabs x 128 contiguous per (tile, k)
    slabs_in = nc.dram_tensor(
        "slabs", [128, 2, NB, 12 * 128], FP8, kind="ExternalInput"
    ).ap()
    acc_out = nc.dram_tensor("acc", [128, ACC_COLS], F32, kind="ExternalOutput").ap()

    with tile.TileContext(nc) as tc, ExitStack() as ctx:
        data = ctx.enter_context(tc.tile_pool(name="data", bufs=1))
        psum = ctx.enter_context(tc.tile_pool(name="psum", bufs=7, space="PSUM"))
        red = ctx.enter_context(tc.tile_pool(name="red", bufs=4))

        # each slab tile is split along k so the DMA stream stays just ahead
        # of matmul consumption; tile A's first two pieces are single kk-pairs
        # so the first matmul starts as early as possible. DMA emission order
        # matches compute order. Triggered from the scalar queue (HWDGE),
        # which finishes its startup ucode load ahead of the sync queue's
        # first DMA slot.
        bounds = {  # (kk_start, kk_end) per piece, in DoubleRow-pair units
            0: [(0, 1), (1, 2)] + [(k, k + 2) for k in range(2, NKP, 2)],
            1: [(k, k + 2) for k in range(0, NKP, 2)],
        }
        pieces = {0: [], 1: []}
        for ti in range(2):
            for k0, k1 in bounds[ti]:
                p = data.tile([128, 2 * (k1 - k0), 12 * 128], FP8, tag=f"t{ti}k{k0}")
                nc.scalar.dma_start(p[:], slabs_in[:, ti, 2 * k0 : 2 * k1, :])
                pieces[ti].append((k0, k1, p))
        acc = data.tile([128, ACC_COLS], F32, tag="acc")

        def piece_for(ti, kk):
            for k0, k1, p in pieces[ti]:
                if k0 <= kk < k1:
                    return p, kk - k0
            raise AssertionError

        col = 0
        acc_done = 0
        for pi, phase in enumerate(PHASES):
            for a in A_ORDER[pi]:
                chains = []
                for g in phase:
                    _, _, js, w = _chain(a, g)
                    jloc = js - 8 * pi  # tile B starts at slab 8
                    ps = psum.tile([128, 512], F32, tag="ps")
                    chains.append((ps, g, jloc, w))
                for kk in range(NKP):
                    # lhsT slab a always lives in tile A
                    lp, lk = piece_for(0, kk)
                    lhsT = lp[:, 2 * lk : 2 * lk + 2, 128 * a : 128 * (a + 1)]
                    rp, rk = piece_for(pi, kk)
                    for ps, g, jloc, w in chains:
                        nc.tensor.matmul(
                            ps[:, : w * 128],
                            lhsT=lhsT,
                            rhs=rp[:, 2 * rk : 2 * rk + 2, 128 * jloc : 128 * (jloc + w)],
                            start=(kk == 0),
                            stop=(kk == NKP - 1),
                            perf_mode=mybir.MatmulPerfMode.DoubleRow,
                        )
                for ps, g, jloc, w in chains:
                    sq = red.tile([128, 512], F32, tag="sq")
                    nc.scalar.activation(
                        sq[:, : w * 128],
                        ps[:, : w * 128],
                        mybir.ActivationFunctionType.Square,
                    )
                    nc.vector.tensor_reduce(
                        acc[:, col : col + w],
                        sq[:, : w * 128].rearrange("p (j c) -> p j c", c=128),
                        axis=mybir.AxisListType.X,
                        op=mybir.AluOpType.add,
                    )
                    col += w
                # in the last phase, ship each a's columns as they finish
                if pi == len(PHASES) - 1:
                    nc.sync.dma_start(acc_out[:, acc_done:col], acc[:, acc_done:col])
                    acc_done = col
            if acc_done < col:
                nc.sync.dma_start(acc_out[:, acc_done:col], acc[:, acc_done:col])
                acc_done = col
    nc.compile()
    return nc


_CACHE = {}


def _get(name, builder):
    if name not in _CACHE:
        _CACHE[name] = builder()
    return _CACHE[name]


def pack_inputs(results, targets):
    """Host: normalize rows, cast to fp8, pack per-core rotated slab arrays.

    Also returns the d=16 cross-gram diagonal square-sum (full f32 GEMM,
    f64 reduce), which the device scheme would otherwise compute twice.
    """
    tn = np.maximum(np.sqrt(np.einsum("ij,ij->i", targets, targets)), EPS_NORM)
    rn = np.maximum(np.sqrt(np.einsum("ij,ij->i", results, results)), EPS_NORM)
    C = np.empty((N, 2 * D), dtype=np.float32)
    np.multiply(targets, (SCALE / tn)[:, None], out=C[:, :D])
    np.multiply(results, (SCALE / rn)[:, None], out=C[:, D:])
    Ct = C[:, :D].reshape(N, 16, 128).transpose(1, 2, 0)
    Cr = C[:, D:].reshape(N, 16, 128).transpose(1, 0, 2)
    cross = np.matmul(Ct, Cr)  # [16, 128, 128] = Tn_i^T @ Rn_i, scaled
    host_sq = float((cross.astype(np.float64) ** 2).sum())
    C8 = C.astype(ml_dtypes.float8_e4m3)
    # [n, col] -> [p, j, k, c] with n = 128k + p, col = 128j + c
    S = C8.reshape(NB, 128, NB, 128).transpose(1, 2, 0, 3)
    in_maps = []
    for c in range(N_CORES):
        jidx = [(4 * c + j) % NB for j in range(JL)]
        rot = S[:, jidx, :, :]  # [p, 20j, k, c]
        tiles = [
            rot[:, sl, :, :].transpose(0, 2, 1, 3).reshape(128, NB, 12 * 128)
            for sl in (TILE_SLABS[0], TILE_SLABS[1])
        ]
        in_maps.append({"slabs": np.ascontiguousarray(np.stack(tiles, axis=1))})
    return in_maps, host_sq


def finish(res, host_sq):
    """Host: weighted f64 reduction of per-block square sums + final sqrt."""
    total = -2.0 * host_sq  # d=16 blocks are all cross-half (sign -1, x2)
    for c in range(N_CORES):
        colsum = res[c]["acc"].astype(np.float64).sum(axis=0)
        col = 0
        for a, g, js, w in CHAINS:
            i_g = 4 * c + a
            for t in range(w):
                j = js + t
                d = j - a
                j_g = (4 * c + j) % NB
                sign = 1.0 if (i_g < 16) == (j_g < 16) else -1.0
                mult = 1.0 if d == 0 else 2.0
                total += sign * mult * colsum[col]
                col += 1
    sq = total / SCALE**4
    return np.float32(np.sqrt(sq * N + EPS_LOSS))


def kernel(results, targets):
    results = np.asarray(results, dtype=np.float32)
    targets = np.asarray(targets, dtype=np.float32)
    in_maps, host_sq = pack_inputs(results, targets)
    nc = _get("gram", build_gram)
    res = run_bass_kernel_spmd(nc, in_maps, list(range(N_CORES))).results
    return finish(res, host_sq)


# revision 26
# speedup vs baseline: 1.0283x; 1.0100x over previous
"""KDLoss kernel for 8 TRN2 NeuronCores — single-launch fp8 symmetric gram.

loss = sqrt(N * || Tn@Tn.T - Rn@Rn.T ||_F^2 + 1e-5), Tn/Rn row-normalized.

Rewrites the Frobenius norm through the stacked matrix C = [Tn | Rn]
(N x 2D) and its symmetric gram M = C^T C (4096 x 4096):

  || Tn Tn^T - Rn Rn^T ||_F^2 = sum_{ij} s_i s_j M_ij^2 ,
  s = (+1 on the Tn half, -1 on the Rn half).

M is symmetric, so the device computes only cyclic block-diagonals
d = 0..15 of the 32x32 grid of 128x128 blocks (512 blocks, vs 768 for
three dense grams). Every core runs the same SPMD program on a rotated
view of the slabs: core c's local slab j holds global slab (4c + j) mod
32, and it computes blocks (a, a+d) for a in 0..3, d in 0..15. The 16
d=16 blocks (the diagonal of the Tn^T Rn cross gram) would be computed
twice under this rotation, so the host does them once in full f32
instead. Host maps each accumulator column back to its global block and
applies the +-1/x2 weights in float64.

The 20 slabs a core needs are shipped as two overlapping SBUF tiles
(slabs 0..11 and 8..19) so every accumulation chain is a full 512-wide
rhs window — 256 uniform DoubleRow matmuls per core, weight loads fully
hidden behind the 2x213ns streams they shadow.

Normalization, the fp8e4 cast (scaled by 16 to center the dynamic
range; end-to-end rel err ~7e-4), slab packing, and the final
weighted-reduce + sqrt run on the host; the device does only the gram
contractions (fp8e4 DoubleRow matmuls into f32 PSUM, ~855 ns per block
at the sustained ~150 TF/s fp8 rate) and per-block square-sums (scalar
Square + vector reduce). Input DMAs stream in k-pieces sized to stay
just ahead of matmul consumption and are triggered from the scalar
queue; the tensor engine starts ~12 us into the NEFF and then runs
gapless (<0.5 us of stalls) until the final drain.
"""

import sys

if "/opt/trn_rl_repo" not in sys.path:
    sys.path.insert(0, "/opt/trn_rl_repo")

from contextlib import ExitStack

import ml_dtypes
import numpy as np

import concourse.bacc as bacc
import concourse.tile as tile
from concourse import mybir
from concourse.bass_utils import run_bass_kernel_spmd

N_CORES = 8
N, D = 4096, 2048
NB = 32                  # slabs of 128 gram columns (2D / 128)
JL = 20                  # local slabs per core: a in 0..3, b = a+d, d <= 15 (+1 pad)
NKP = 16                 # 32 k-tiles as 16 DoubleRow pairs
SCALE = 16.0             # pre-quantization scale to center fp8e4 range
EPS_NORM = 1e-12
EPS_LOSS = 1e-05
F32 = mybir.dt.float32
FP8 = mybir.dt.float8e4

# chain layout: (a, g, jstart, width). The device covers diagonals d = 0..15
# only (the 16 d=16 blocks — the diagonal of the Tn^T Rn cross gram — are
# cheaper to do exactly on host than to compute twice under the SPMD
# rotation), so per a the rhs windows are [a+4g .. a+4g+3]: every chain is a
# full 512-wide matmul stream. Windows for g<2 live in slab-tile A (slabs
# 0..11), for g>=2 in tile B (slabs 8..19); the 4-slab overlap is loaded
# twice, which is cheap next to the matmul time it saves. Chains of one
# (phase, a) interleave their matmuls so consecutive matmuls share the same
# stationary lhsT slab.
PHASES = [(0, 1), (2, 3)]


def _chain(a, g):
    return (a, g, a + 4 * g, 4)


A_ORDER = {0: (0, 1, 2, 3), 1: (3, 2, 1, 0)}
CHAINS = [
    _chain(a, g) for pi, phase in enumerate(PHASES) for a in A_ORDER[pi] for g in phase
]
ACC_COLS = sum(w for _, _, _, w in CHAINS)  # 64
TILE_SLABS = {0: list(range(12)), 1: list(range(8, 20))}  # A, B


def build_gram():
    nc = bacc.Bacc("TRN2", target_bir_lowering=False, num_devices=N_CORES)
    # (p, tile, k, jc): lane p reads 12 slabs x 128 contiguous per (tile, k)
    slabs_in = nc.dram_tensor(
        "slabs", [128, 2, NB, 12 * 128], FP8, kind="ExternalInput"
    ).ap()
    acc_out = nc.dram_tensor("acc", [128, ACC_COLS], F32, kind="ExternalOutput").ap()

    with tile.TileContext(nc) as tc, ExitStack() as ctx:
        data = ctx.enter_context(tc.tile_pool(name="data", bufs=1))
        psum = ctx.enter_context(tc.tile_pool(name="psum", bufs=7, space="PSUM"))
        red = ctx.enter_context(tc.tile_pool(name="red", bufs=4))

        # each slab tile is split along k so the DMA stream stays just ahead
        # of matmul consumption; tile A's first two pieces are single kk-pairs
        # so the first matmul starts as early as possible. DMA emission order
        # matches compute order. Triggered from the scalar queue (HWDGE),
        # which finishes its startup ucode load ahead of the sync queue's
        # first DMA slot.
        bounds = {  # (kk_start, kk_end) per piece, in DoubleRow-pair units
            0: [(0, 1), (1, 2)] + [(k, k + 2) for k in range(2, NKP, 2)],
            1: [(k, k + 2) for k in range(0, NKP, 2)],
        }
        pieces = {0: [], 1: []}
        for ti in range(2):
            for k0, k1 in bounds[ti]:
                p = data.tile([128, 2 * (k1 - k0), 12 * 128], FP8, tag=f"t{ti}k{k0}")
                nc.scalar.dma_start(p[:], slabs_in[:, ti, 2 * k0 : 2 * k1, :])
                pieces[ti].append((k0, k1, p))
        acc = data.tile([128, ACC_COLS], F32, tag="acc")

        def piece_for(ti, kk):
            for k0, k1, p in pieces[ti]:
                if k0 <= kk < k1:
                    return p, kk - k0
            raise AssertionError

        col = 0
        acc_done = 0
        for pi, phase in enumerate(PHASES):
            for a in A_ORDER[pi]:
                chains = []
                for g in phase:
                    _, _, js, w = _chain(a, g)
                    jloc = js - 8 * pi  # tile B starts at slab 8
                    ps = psum.tile([128, 512], F32, tag="ps")
                    chains.append((ps, g, jloc, w))
                for kk in range(NKP):
                    # lhsT slab a always lives in tile A
                    lp, lk = piece_for(0, kk)
                    lhsT = lp[:, 2 * lk : 2 * lk + 2, 128 * a : 128 * (a + 1)]
                    rp, rk = piece_for(pi, kk)
                    for ps, g, jloc, w in chains:
                        nc.tensor.matmul(
                            ps[:, : w * 128],
                            lhsT=lhsT,
                            rhs=rp[:, 2 * rk : 2 * rk + 2, 128 * jloc : 128 * (jloc + w)],
                            start=(kk == 0),
                            stop=(kk == NKP - 1),
                            perf_mode=mybir.MatmulPerfMode.DoubleRow,
                        )
                for ps, g, jloc, w in chains:
                    sq = red.tile([128, 512], F32, tag="sq")
                    nc.scalar.activation(
                        sq[:, : w * 128],
                        ps[:, : w * 128],
                        mybir.ActivationFunctionType.Square,
                    )
                    nc.vector.tensor_reduce(
                        acc[:, col : col + w],
                        sq[:, : w * 128].rearrange("p (j c) -> p j c", c=128),
                        axis=mybir.AxisListType.X,
                        op=mybir.AluOpType.add,
                    )
                    col += w
                # in the last phase, ship each a's columns as they finish
                if pi == len(PHASES) - 1:
                    nc.sync.dma_start(acc_out[:, acc_done:col], acc[:, acc_done:col])
                    acc_done = col
            if acc_done < col:
                nc.sync.dma_start(acc_out[:, acc_done:col], acc[:, acc_done:col])
                acc_done = col
    nc.compile()
    return nc


_CACHE = {}


def _get(name, builder):
    if name not in _CACHE:
        _CACHE[name] = builder()
    return _CACHE[name]


def pack_inputs(results, targets):
    """Host: normalize rows, cast to fp8, pack per-core rotated slab arrays.

    Also returns the d=16 cross-gram diagonal square-sum (full f32 GEMM,
    f64 reduce), which the device scheme would otherwise compute twice.
    """
    tn = np.maximum(np.sqrt(np.einsum("ij,ij->i", targets, targets)), EPS_NORM)
    rn = np.maximum(np.sqrt(np.einsum("ij,ij->i", results, results)), EPS_NORM)
    C = np.empty((N, 2 * D), dtype=np.float32)
    np.multiply(targets, (SCALE / tn)[:, None], out=C[:, :D])
    np.multiply(results, (SCALE / rn)[:, None], out=C[:, D:])
    Ct = C[:, :D].reshape(N, 16, 128).transpose(1, 2, 0)
    Cr = C[:, D:].reshape(N, 16, 128).transpose(1, 0, 2)
    cross = np.matmul(Ct, Cr)  # [16, 128, 128] = Tn_i^T @ Rn_i, scaled
    host_sq = float((cross.astype(np.float64) ** 2).sum())
    C8 = C.astype(ml_dtypes.float8_e4m3)
    # [n, col] -> [p, j, k, c] with n = 128k + p, col = 128j + c
    S = C8.reshape(NB, 128, NB, 128).transpose(1, 2, 0, 3)
    in_maps = []
    for c in range(N_CORES):
        jidx = [(4 * c + j) % NB for j in range(JL)]
        rot = S[:, jidx, :, :]  # [p, 20j, k, c]
        tiles = [
            rot[:, sl, :, :].transpose(0, 2, 1, 3).reshape(128, NB, 12 * 128)
            for sl in (TILE_SLABS[0], TILE_SLABS[1])
        ]
        in_maps.append({"slabs": np.ascontiguousarray(np.stack(tiles, axis=1))})
    return in_maps, host_sq


def finish(res, host_sq):
    """Host: weighted f64 reduction of per-block square sums + final sqrt."""
    total = -2.0 * host_sq  # d=16 blocks are all cross-half (sign -1, x2)
    for c in range(N_CORES):
        colsum = res[c]["acc"].astype(np.float64).sum(axis=0)
        col = 0
        for a, g, js, w in CHAINS:
            i_g = 4 * c + a
            for t in range(w):
                j = js + t
                d = j - a
                j_g = (4 * c + j) % NB
                sign = 1.0 if (i_g < 16) == (j_g < 16) else -1.0
                mult = 1.0 if d == 0 else 2.0
                total += sign * mult * colsum[col]
                col += 1
    sq = total / SCALE**4
    return np.float32(np.sqrt(sq * N + EPS_LOSS))


def kernel(results, targets):
    results = np.asarray(results, dtype=np.float32)
    targets = np.asarray(targets, dtype=np.float32)
    in_maps, host_sq = pack_inputs(results, targets)
    nc = _get("gram", build_gram)
    res = run_bass_kernel_spmd(nc, in_maps, list(range(N_CORES))).results
    return finish(res, host_sq)


# revision 27
# speedup vs baseline: 1.0441x; 1.0154x over previous
"""KDLoss kernel for 8 TRN2 NeuronCores — single-launch fp8 symmetric gram.

loss = sqrt(N * || Tn@Tn.T - Rn@Rn.T ||_F^2 + 1e-5), Tn/Rn row-normalized.

Rewrites the Frobenius norm through the stacked matrix C = [Tn | Rn]
(N x 2D) and its symmetric gram M = C^T C (4096 x 4096):

  || Tn Tn^T - Rn Rn^T ||_F^2 = sum_{ij} s_i s_j M_ij^2 ,
  s = (+1 on the Tn half, -1 on the Rn half).

M is symmetric, so the device computes only cyclic block-diagonals
d = 0..15 of the 32x32 grid of 128x128 blocks (512 blocks, vs 768 for
three dense grams). Every core runs the same SPMD program on a rotated
view of the slabs: core c's local slab j holds global slab (4c + j) mod
32, and it computes blocks (a, a+d) for a in 0..3, d in 0..15. The 16
d=16 blocks (the diagonal of the Tn^T Rn cross gram) would be computed
twice under this rotation, so the host does them once in full f32
instead. Host maps each accumulator column back to its global block and
applies the +-1/x2 weights in float64.

The 20 slabs a core needs are shipped as two overlapping SBUF tiles
(slabs 0..11 and 8..19) so every accumulation chain is a full 512-wide
rhs window — 256 uniform DoubleRow matmuls per core, weight loads fully
hidden behind the 2x213ns streams they shadow.

Normalization, the fp8e4 cast (scaled by 16 to center the dynamic
range; end-to-end rel err ~7e-4), slab packing, and the final
weighted-reduce + sqrt run on the host; the device does only the gram
contractions (fp8e4 DoubleRow matmuls into f32 PSUM, ~855 ns per block
at the sustained ~150 TF/s fp8 rate) and per-block square-sums (scalar
Square + vector reduce). Input DMAs stream in k-pieces sized to stay
just ahead of matmul consumption and are triggered from the scalar
queue; the tensor engine starts ~12 us into the NEFF and then runs
gapless (<0.5 us of stalls) until the final drain.
"""

import sys

if "/opt/trn_rl_repo" not in sys.path:
    sys.path.insert(0, "/opt/trn_rl_repo")

from contextlib import ExitStack

import ml_dtypes
import numpy as np

import concourse.bacc as bacc
import concourse.tile as tile
from concourse import mybir
from concourse.bass_utils import run_bass_kernel_spmd

N_CORES = 8
N, D = 4096, 2048
NB = 32                  # slabs of 128 gram columns (2D / 128)
JL = 20                  # local slabs per core: a in 0..3, b = a+d, d <= 15 (+1 pad)
NKP = 16                 # 32 k-tiles as 16 DoubleRow pairs
SCALE = 16.0             # pre-quantization scale to center fp8e4 range
EPS_NORM = 1e-12
EPS_LOSS = 1e-05
F32 = mybir.dt.float32
FP8 = mybir.dt.float8e4

# chain layout: (a, g, jstart, width). The device covers diagonals d = 0..15
# only (the 16 d=16 blocks — the diagonal of the Tn^T Rn cross gram — are
# cheaper to do exactly on host than to compute twice under the SPMD
# rotation), so per a the rhs windows are [a+4g .. a+4g+3]: every chain is a
# full 512-wide matmul stream. Windows for g<2 live in slab-tile A (slabs
# 0..11), for g>=2 in tile B (slabs 8..19); the 4-slab overlap is loaded
# twice, which is cheap next to the matmul time it saves. Chains of one
# (phase, a) interleave their matmuls so consecutive matmuls share the same
# stationary lhsT slab.
PHASES = [(0, 1), (2, 3)]


def _chain(a, g):
    return (a, g, a + 4 * g, 4)


A_ORDER = {0: (0, 1, 2, 3), 1: (3, 2, 1, 0)}
CHAINS = [
    _chain(a, g) for pi, phase in enumerate(PHASES) for a in A_ORDER[pi] for g in phase
]
ACC_COLS = sum(w for _, _, _, w in CHAINS)  # 64
TILE_SLABS = {0: list(range(12)), 1: list(range(8, 20))}  # A, B


def build_gram():
    nc = bacc.Bacc("TRN2", target_bir_lowering=False, num_devices=N_CORES)
    # (p, tile, k, jc): lane p reads 12 slabs x 128 contiguous per (tile, k)
    slabs_in = nc.dram_tensor(
        "slabs", [128, 2, NB, 12 * 128], FP8, kind="ExternalInput"
    ).ap()
    acc_out = nc.dram_tensor("acc", [128, ACC_COLS], F32, kind="ExternalOutput").ap()

    with tile.TileContext(nc) as tc, ExitStack() as ctx:
        data = ctx.enter_context(tc.tile_pool(name="data", bufs=1))
        psum = ctx.enter_context(tc.tile_pool(name="psum", bufs=7, space="PSUM"))
        red = ctx.enter_context(tc.tile_pool(name="red", bufs=4))

        # each slab tile is split along k so the DMA stream stays just ahead
        # of matmul consumption; tile A's first two pieces are single kk-pairs
        # so the first matmul starts as early as possible. DMA emission order
        # matches compute order. Triggered from the scalar queue (HWDGE),
        # which finishes its startup ucode load ahead of the sync queue's
        # first DMA slot.
        bounds = {  # (kk_start, kk_end) per piece, in DoubleRow-pair units
            0: [(0, 1), (1, 2)] + [(k, k + 2) for k in range(2, NKP, 2)],
            1: [(k, k + 2) for k in range(0, NKP, 2)],
        }
        pieces = {0: [], 1: []}
        for ti in range(2):
            for k0, k1 in bounds[ti]:
                p = data.tile([128, 2 * (k1 - k0), 12 * 128], FP8, tag=f"t{ti}k{k0}")
                nc.scalar.dma_start(p[:], slabs_in[:, ti, 2 * k0 : 2 * k1, :])
                pieces[ti].append((k0, k1, p))
        acc = data.tile([128, ACC_COLS], F32, tag="acc")

        def piece_for(ti, kk):
            for k0, k1, p in pieces[ti]:
                if k0 <= kk < k1:
                    return p, kk - k0
            raise AssertionError

        def drain(ps, w, col):
            sq = red.tile([128, 512], F32, tag="sq")
            nc.scalar.activation(
                sq[:, : w * 128], ps[:, : w * 128], mybir.ActivationFunctionType.Square
            )
            nc.vector.tensor_reduce(
                acc[:, col : col + w],
                sq[:, : w * 128].rearrange("p (j c) -> p j c", c=128),
                axis=mybir.AxisListType.X,
                op=mybir.AluOpType.add,
            )

        def matmuls(ps, pi, a, chain_list, kk):
            lp, lk = piece_for(0, kk)  # lhsT slab a always lives in tile A
            lhsT = lp[:, 2 * lk : 2 * lk + 2, 128 * a : 128 * (a + 1)]
            rp, rk = piece_for(pi, kk)
            for ps_, g, jloc, w in chain_list:
                nc.tensor.matmul(
                    ps_[:, : w * 128],
                    lhsT=lhsT,
                    rhs=rp[:, 2 * rk : 2 * rk + 2, 128 * jloc : 128 * (jloc + w)],
                    start=(kk == 0),
                    stop=(kk == NKP - 1),
                    perf_mode=mybir.MatmulPerfMode.DoubleRow,
                )

        col = 0
        acc_done = 0
        n_phases = len(PHASES)
        for pi, phase in enumerate(PHASES):
            order = A_ORDER[pi]
            for ai, a in enumerate(order):
                last_group = pi == n_phases - 1 and ai == len(order) - 1
                chains = []
                for g in phase:
                    _, _, js, w = _chain(a, g)
                    jloc = js - 8 * pi  # tile B starts at slab 8
                    ps = psum.tile([128, 512], F32, tag="ps")
                    chains.append((ps, g, jloc, w))
                if last_group:
                    # run the final chains sequentially so the first chain's
                    # drain and acc columns hide under the second chain's
                    # matmuls — only one drain stays on the critical tail
                    for ps, g, jloc, w in chains:
                        for kk in range(NKP):
                            matmuls(ps, pi, a, [(ps, g, jloc, w)], kk)
                        drain(ps, w, col)
                        col += w
                        nc.sync.dma_start(acc_out[:, acc_done:col], acc[:, acc_done:col])
                        acc_done = col
                else:
                    for kk in range(NKP):
                        matmuls(None, pi, a, chains, kk)
                    for ps, g, jloc, w in chains:
                        drain(ps, w, col)
                        col += w
                    # in the last phase, ship each a's columns as they finish
                    if pi == n_phases - 1:
                        nc.sync.dma_start(acc_out[:, acc_done:col], acc[:, acc_done:col])
                        acc_done = col
            if acc_done < col:
                nc.sync.dma_start(acc_out[:, acc_done:col], acc[:, acc_done:col])
                acc_done = col
    nc.compile()
    return nc


_CACHE = {}


def _get(name, builder):
    if name not in _CACHE:
        _CACHE[name] = builder()
    return _CACHE[name]


def pack_inputs(results, targets):
    """Host: normalize rows, cast to fp8, pack per-core rotated slab arrays.

    Also returns the d=16 cross-gram diagonal square-sum (full f32 GEMM,
    f64 reduce), which the device scheme would otherwise compute twice.
    """
    tn = np.maximum(np.sqrt(np.einsum("ij,ij->i", targets, targets)), EPS_NORM)
    rn = np.maximum(np.sqrt(np.einsum("ij,ij->i", results, results)), EPS_NORM)
    C = np.empty((N, 2 * D), dtype=np.float32)
    np.multiply(targets, (SCALE / tn)[:, None], out=C[:, :D])
    np.multiply(results, (SCALE / rn)[:, None], out=C[:, D:])
    Ct = C[:, :D].reshape(N, 16, 128).transpose(1, 2, 0)
    Cr = C[:, D:].reshape(N, 16, 128).transpose(1, 0, 2)
    cross = np.matmul(Ct, Cr)  # [16, 128, 128] = Tn_i^T @ Rn_i, scaled
    host_sq = float((cross.astype(np.float64) ** 2).sum())
    C8 = C.astype(ml_dtypes.float8_e4m3)
    # [n, col] -> [p, j, k, c] with n = 128k + p, col = 128j + c
    S = C8.reshape(NB, 128, NB, 128).transpose(1, 2, 0, 3)
    in_maps = []
    for c in range(N_CORES):
        jidx = [(4 * c + j) % NB for j in range(JL)]
        rot = S[:, jidx, :, :]  # [p, 20j, k, c]
        tiles = [
            rot[:, sl, :, :].transpose(0, 2, 1, 3).reshape(128, NB, 12 * 128)
            for sl in (TILE_SLABS[0], TILE_SLABS[1])
        ]
        in_maps.append({"slabs": np.ascontiguousarray(np.stack(tiles, axis=1))})
    return in_maps, host_sq


def finish(res, host_sq):
    """Host: weighted f64 reduction of per-block square sums + final sqrt."""
    total = -2.0 * host_sq  # d=16 blocks are all cross-half (sign -1, x2)
    for c in range(N_CORES):
        colsum = res[c]["acc"].astype(np.float64).sum(axis=0)
        col = 0
        for a, g, js, w in CHAINS:
            i_g = 4 * c + a
            for t in range(w):
                j = js + t
                d = j - a
                j_g = (4 * c + j) % NB
                sign = 1.0 if (i_g < 16) == (j_g < 16) else -1.0
                mult = 1.0 if d == 0 else 2.0
                total += sign * mult * colsum[col]
                col += 1
    sq = total / SCALE**4
    return np.float32(np.sqrt(sq * N + EPS_LOSS))


def kernel(results, targets):
    results = np.asarray(results, dtype=np.float32)
    targets = np.asarray(targets, dtype=np.float32)
    in_maps, host_sq = pack_inputs(results, targets)
    nc = _get("gram", build_gram)
    res = run_bass_kernel_spmd(nc, in_maps, list(range(N_CORES))).results
    return finish(res, host_sq)
